# revision 4
# baseline (speedup 1.0000x reference)
"""Cross-attention kernel for Trainium2, 8 NeuronCores.

Reference computation (per batch b, with n = h*w = 9216, c = 128, cq = 16):
    q  = (w_q @ y_b)                       # [cq, n]   (used transposed)
    k  = (w_k @ y_b)                       # [cq, n]
    s  = q^T @ k                           # [n, n]    scores
    m  = softmax(s, axis=-1)
    v  = (w_v @ x_b)                       # [c, n]
    out = v @ m^T                          # [c, n]

Sharding: 8 cores = (batch b in {0,1}) x (query block qb in {0..3}, 2304
queries each). Each core sees all 9216 keys.

v2 design (engine-balanced flash loop), changes vs the v1 baseline:
- Score matmuls are 4-way tile_position row-packed for ALL windows (v1 only
  packed the tail): windows are 256 queries wide, groups are 4 key chunks,
  st tiles [128, 4, 256].  PE score cost per group drops 2x.
- The softmax denominator is mostly OFF the PE: exp tiles (fp16) are
  accumulated elementwise on the DVE (2x fp16 mode) into a per-window acc
  tile; only XPE groups/window keep PE ones-matmuls, chosen to balance
  PE vs DVE occupancy.  (In v1 the den ones-matmuls cost as much PE time
  as the feat matmuls.)
- exp output is fp16 (2x ACT throughput, measured in v1), which also makes
  the feat/den moving operands fp16 (1 cycle/col at any width).  VT stays
  f32r as the feat stationary operand (mixed dtypes are allowed unless one
  side is plain f32), so V keeps full precision; measured error ~4e-3.
- VT prep uses fp16 wv as the moving operand: 128 cols x 1 cyc/col, 4x
  cheaper than v1's plain-f32 matmuls.
- Per window: den partials -> one PSUM row (ones-matmuls, fp16 moving),
  reciprocal on DVE, broadcast back to [128, QW] via a rank-1 PE matmul
  into the same aux PSUM bank, final feat*recip on DVE, DMA out.
- PSUM budget exactly 8 banks: 3 st slots x 2 + feat + aux.
- f32->f32r/fp16 rounds run on ACT, PSUM evacuations on DVE, splitting the
  prep work across both engines.
"""

import numpy as np

import concourse.bacc as bacc
import concourse.tile as tile
from concourse import mybir

f32 = mybir.dt.float32
f32r = mybir.dt.float32r
fp16 = mybir.dt.float16

P = 128          # partitions / channels
NK = 9216        # keys (h*w)
NQ = 2304        # queries per core
KC = NK // P     # 72 key chunks of 128
CQ = 16          # query/key projection dim
QW = 256         # query window width (fp32r fast-path floor)
NW = NQ // QW    # 9 windows
GSZ = 4          # key chunks per group (4-way tile_position volley)
NG = KC // GSZ   # 18 groups per window
LAG = 3          # groups of software-pipelining lag for feat/den
XPE = 3          # groups per window whose denominator runs on the PE

_CACHE = {}


def _build():
    nc = bacc.Bacc(trn_type="TRN2", target_bir_lowering=False, debug=False)
    y = nc.dram_tensor("y", [P, NK], f32, kind="ExternalInput")
    yq = nc.dram_tensor("yq", [P, NQ], f32, kind="ExternalInput")
    x = nc.dram_tensor("x", [P, NK], f32, kind="ExternalInput")
    # w_q^T / w_k^T replicated into four 32-row strips ([wT,0,wT,0,wT,0,wT])
    # so the score matmuls can run 4-way row-packed via tile_position.
    wq = nc.dram_tensor("wq", [P, 112], f32, kind="ExternalInput")
    wk = nc.dram_tensor("wk", [P, 112], f32, kind="ExternalInput")
    wv = nc.dram_tensor("wv", [P, P], f32, kind="ExternalInput")    # w_v^T
    o = nc.dram_tensor("o", [P, NQ], f32, kind="ExternalOutput")

    Exp = mybir.ActivationFunctionType.Exp
    Copy = mybir.ActivationFunctionType.Copy

    with tile.TileContext(nc) as tc:
        with (
            tc.tile_pool(name="const", bufs=1) as const,
            tc.tile_pool(name="big", bufs=1) as big,
            tc.tile_pool(name="xs", bufs=2) as xs,
            tc.tile_pool(name="ps", bufs=3, space="PSUM") as ps,
            tc.tile_pool(name="featp", bufs=1, space="PSUM") as featp,
            tc.tile_pool(name="auxp", bufs=1, space="PSUM") as auxp,
            tc.tile_pool(name="ep", bufs=LAG + 2) as ep,
            tc.tile_pool(name="accp", bufs=2) as accp,
            tc.tile_pool(name="op", bufs=2) as op,
            tc.tile_pool(name="small", bufs=2) as small,
        ):
            # ---- constants ----
            wq_sb = const.tile([P, 112], f32, name="wq_sb")
            nc.sync.dma_start(wq_sb, wq.ap())
            wk_sb = const.tile([P, 112], f32, name="wk_sb")
            nc.sync.dma_start(wk_sb, wk.ap())
            wv_sb = const.tile([P, P], f32, name="wv_sb")
            nc.sync.dma_start(wv_sb, wv.ap())
            wvb = const.tile([P, P], fp16, name="wvb")
            nc.vector.tensor_copy(wvb, wv_sb)
            wkr = const.tile([P, 112], f32r, name="wkr")
            nc.vector.tensor_copy(wkr, wk_sb)
            wqr = const.tile([P, 112], f32r, name="wqr")
            nc.vector.tensor_copy(wqr, wq_sb)
            ones_col = const.tile([P, 1], fp16, name="ones_col")
            nc.vector.memset(ones_col, 1.0)
            ones1f = const.tile([1, P], f32, name="ones1f")
            nc.vector.memset(ones1f, 1.0)
            ones1 = const.tile([1, P], f32r, name="ones1")
            nc.vector.tensor_copy(ones1, ones1f)

            K_sb = big.tile([112, NK], f32r, name="K_sb")
            Q_sb = big.tile([112, NQ], f32r, name="Q_sb")
            VT = big.tile([P, NK], fp16, name="VT")

            # ---- prep ----
            # yq first (the whole Q projection gates the first score matmul),
            # then y/x chunks interleaved.  f32->f32r rounds go on ACT, the
            # PSUM evacuations on DVE.
            def emit_proj(i):
                src = y.ap()[:, i * NQ : (i + 1) * NQ] if i < 4 else yq.ap()
                yst = xs.tile([P, NQ], f32, tag="yst", name=f"yst{i}")
                nc.sync.dma_start(yst, src)
                yr = xs.tile([P, NQ], f32r, tag="yr", name=f"yr{i}")
                nc.scalar.activation(yr, yst, Copy)
                wr = wkr if i < 4 else wqr
                dst = K_sb if i < 4 else Q_sb
                dof = i * NQ if i < 4 else 0
                for t, qs in enumerate(range(0, NQ, 512)):
                    qw = min(512, NQ - qs)
                    kp = ps.tile([112, qw], f32, tag="st", name=f"kp{i}_{t}")
                    nc.tensor.matmul(kp, wr, yr[:, qs : qs + qw], start=True, stop=True)
                    nc.vector.tensor_copy(dst[:, dof + qs : dof + qs + qw], kp)

            def emit_vt(i):
                # vT chunks [128 keys, 128 c] = x_chunk^T @ w_v^T with fp16
                # moving wv (1 cyc/col); evacuate eight chunks per DVE copy.
                xt = xs.tile([P, NQ], f32, tag="xt", name=f"xt{i}")
                nc.sync.dma_start(xt, x.ap()[:, i * NQ : (i + 1) * NQ])
                xr = xs.tile([P, NQ], fp16, tag="xr", name=f"xr{i}")
                nc.scalar.activation(xr, xt, Copy)
                nkc = NQ // P  # 18
                for b0 in range(0, nkc, 8):
                    nb = min(8, nkc - b0)
                    vp = ps.tile([P, nb * P], f32, tag="st", name=f"vp{i}_{b0}")
                    for t in range(b0, b0 + nb):
                        nc.tensor.matmul(
                            vp[:, (t - b0) * P : (t - b0 + 1) * P],
                            xr[:, t * P : (t + 1) * P],
                            wvb,
                            start=True,
                            stop=True,
                        )
                    kc0 = i * nkc + b0
                    nc.vector.tensor_copy(VT[:, kc0 * P : (kc0 + nb) * P], vp)

            emit_proj(4)  # yq -> Q_sb
            for i in range(4):
                emit_proj(i)
                emit_vt(i)

            # ---- main flash loop, software-pipelined ----
            # The PE engine queue is in-order: feat/den matmuls are emitted
            # LAG groups behind their score matmuls so the PE never waits on
            # the exp of the group it just scored.
            groups = [(w, g) for w in range(NW) for g in range(NG)]
            et_tiles = {}
            acc_tiles = {}
            feat_tiles = {}
            aux_tiles = {}

            def emit_st(w, g):
                ws = w * QW
                st = ps.tile([P, GSZ, QW], f32, tag="st", name=f"st{w}_{g}")
                for j in range(GSZ):
                    kc = GSZ * g + j
                    nc.tensor.matmul(
                        st[:, j, :],
                        K_sb[32 * j : 32 * j + CQ, kc * P : (kc + 1) * P],
                        Q_sb[32 * j : 32 * j + CQ, ws : ws + QW],
                        start=True,
                        stop=True,
                        tile_position=(32 * j, 0),
                    )
                et = ep.tile([P, GSZ, QW], fp16, tag="e", name=f"e{w}_{g}")
                nc.scalar.activation(et, st, Exp)
                et_tiles[(w, g)] = et
                # DVE side of the denominator: elementwise accumulate the exp
                # tiles (2x fp16 DVE mode); the last XPE groups are left for
                # PE ones-matmuls at feat time to balance the two engines.
                if g < NG - XPE:
                    if g == 0:
                        acc = accp.tile(
                            [P, GSZ, QW], fp16, tag="acc", name=f"acc{w}"
                        )
                        acc_tiles[w] = acc
                        nc.vector.tensor_copy(acc, et)
                    else:
                        nc.vector.tensor_add(acc_tiles[w], acc_tiles[w], et)

            def emit_fd(w, g):
                ws = w * QW
                if g == 0:
                    feat_tiles[w] = featp.tile(
                        [P, QW], f32, tag="feat", name=f"feat{w}"
                    )
                feat_ps = feat_tiles[w]
                et = et_tiles.pop((w, g))
                for j in range(GSZ):
                    kc = GSZ * g + j
                    nc.tensor.matmul(
                        feat_ps,
                        VT[:, kc * P : (kc + 1) * P],
                        et[:, j, :],
                        start=(kc == 0),
                        stop=(kc == KC - 1),
                    )
                if g >= NG - XPE:
                    if g == NG - XPE:
                        aux_tiles[w] = auxp.tile(
                            [P, QW], f32, tag="aux", name=f"aux{w}"
                        )
                    aux = aux_tiles[w]
                    for j in range(GSZ):
                        nc.tensor.matmul(
                            aux[0:1, :],
                            ones_col,
                            et[:, j, :],
                            start=(g == NG - XPE and j == 0),
                            stop=False,
                        )
                if g == NG - 1:
                    aux = aux_tiles.pop(w)
                    acc = acc_tiles.pop(w)
                    for j in range(GSZ):
                        nc.tensor.matmul(
                            aux[0:1, :],
                            ones_col,
                            acc[:, j, :],
                            start=False,
                            stop=(j == GSZ - 1),
                        )
                    rec = small.tile([1, QW], f32r, tag="rec", name=f"rec{w}")
                    with nc.allow_low_precision("f32r recip feeds an fp32r matmul"):
                        nc.vector.reciprocal(rec, aux[0:1, :])
                    # evacuate feat on ACT (frees the feat PSUM bank for the
                    # next window and keeps the final mul to one PSUM operand)
                    feat_sb = small.tile([P, QW], f32, tag="fsb", name=f"fsb{w}")
                    nc.scalar.activation(feat_sb, feat_tiles.pop(w), Copy)
                    # broadcast 1/den to all 128 partitions via a rank-1
                    # matmul into the same aux bank (freed by the recip read)
                    nc.tensor.matmul(aux, ones1, rec, start=True, stop=True)
                    o_sb = op.tile([P, QW], f32, tag="o", name=f"o{w}")
                    nc.vector.tensor_mul(o_sb, feat_sb, aux)
                    nc.sync.dma_start(o.ap()[:, ws : ws + QW], o_sb)

            for idx in range(len(groups) + LAG):
                if idx < len(groups):
                    emit_st(*groups[idx])
                if idx >= LAG:
                    emit_fd(*groups[idx - LAG])

    nc.compile()
    return nc


def _get_runner():
    """Build the Bass module once and wrap it in a cached sharded jax callable.

    Mirrors concourse.bass2jax.run_bass_via_pjrt (the @via_axon execution
    path) but caches the jitted executable so repeated kernel() calls do not
    re-trace/re-compile.
    """
    if "runner" in _CACHE:
        return _CACHE["runner"]

    import jax
    from jax.experimental.shard_map import shard_map
    from jax.sharding import Mesh, PartitionSpec

    from concourse import bass2jax, mybir as _mybir

    bass2jax.install_neuronx_cc_hook()
    nc = _build()

    partition_name = nc.partition_id_tensor.name if nc.partition_id_tensor else None
    in_names, out_names, out_avals = [], [], []
    for alloc in nc.m.functions[0].allocations:
        if not isinstance(alloc, _mybir.MemoryLocationSet):
            continue
        name = alloc.memorylocations[0].name
        if alloc.kind == "ExternalInput":
            if name != partition_name:
                in_names.append(name)
        elif alloc.kind == "ExternalOutput":
            out_names.append(name)
            out_avals.append(
                jax.core.ShapedArray(
                    tuple(alloc.tensor_shape), _mybir.dt.np(alloc.dtype)
                )
            )
    n_params = len(in_names)
    all_in_names = in_names + out_names
    if partition_name is not None:
        all_in_names.append(partition_name)
    donate = tuple(range(n_params, n_params + len(out_names)))

    def _body(*args):
        operands = list(args)
        if partition_name is not None:
            operands.append(bass2jax.partition_id_tensor())
        outs = bass2jax._bass_exec_p.bind(
            *operands,
            out_avals=tuple(out_avals),
            in_names=tuple(all_in_names),
            out_names=tuple(out_names),
            lowering_input_output_aliases=(),
            sim_require_finite=True,
            sim_require_nnan=True,
            nc=nc,
        )
        return tuple(outs)

    devices = jax.devices()[:8]
    mesh = Mesh(np.asarray(devices), ("core",))
    in_specs = (PartitionSpec("core"),) * (n_params + len(out_names))
    out_specs = (PartitionSpec("core"),) * len(out_names)
    smapped = shard_map(
        _body, mesh=mesh, in_specs=in_specs, out_specs=out_specs, check_rep=False
    )
    sharded = jax.jit(smapped, donate_argnums=donate, keep_unused=True)

    out_shapes = [tuple(a.shape) for a in out_avals]
    out_dtypes = [a.dtype for a in out_avals]
    runner = {
        "fn": sharded,
        "smapped": smapped,
        "n_params": n_params,
        "in_names": in_names,
        "out_names": out_names,
        "out_shapes": out_shapes,
        "out_dtypes": out_dtypes,
        "nc": nc,
    }
    _CACHE["runner"] = runner
    return runner


def _run(in_maps):
    r = _get_runner()
    concat_in = [
        np.concatenate([np.asarray(m[name]) for m in in_maps], axis=0)
        for name in r["in_names"]
    ]
    concat_zeros = [
        np.zeros((8 * s[0], *s[1:]), d)
        for s, d in zip(r["out_shapes"], r["out_dtypes"])
    ]
    out_arrs = r["fn"](*concat_in, *concat_zeros)
    return [
        {
            name: np.asarray(out_arrs[i]).reshape(8, *r["out_shapes"][i])[c]
            for i, name in enumerate(r["out_names"])
        }
        for c in range(8)
    ]


def _make_in_maps(x, y, w_q, w_k, w_v):
    x = np.ascontiguousarray(np.asarray(x, dtype=np.float32))
    y = np.ascontiguousarray(np.asarray(y, dtype=np.float32))
    bz, c, h, w = x.shape
    n = h * w
    xf = x.reshape(bz, c, n)
    yf = y.reshape(bz, c, n)
    wqT = np.asarray(w_q, dtype=np.float32).T  # [c, cq]
    wkT = np.asarray(w_k, dtype=np.float32).T
    z = np.zeros((c, 32 - CQ), np.float32)
    wq2 = np.ascontiguousarray(
        np.concatenate([wqT, z, wqT, z, wqT, z, wqT], axis=1)
    )  # [c, 112]
    wk2 = np.ascontiguousarray(np.concatenate([wkT, z, wkT, z, wkT, z, wkT], axis=1))
    wvT = np.ascontiguousarray(np.asarray(w_v, dtype=np.float32).T)  # [c, c]
    in_maps = []
    for cid in range(8):
        b, qb = divmod(cid, 4)
        in_maps.append(
            {
                "y": np.ascontiguousarray(yf[b]),
                "yq": np.ascontiguousarray(yf[b][:, qb * NQ : (qb + 1) * NQ]),
                "x": np.ascontiguousarray(xf[b]),
                "wq": wq2,
                "wk": wk2,
                "wv": wvT,
            }
        )
    return in_maps


def kernel(x, y, w_q, w_k, w_v):
    bz, c, h, w = np.asarray(x).shape
    n = h * w
    results = _run(_make_in_maps(x, y, w_q, w_k, w_v))
    feat = np.empty((bz, c, n), dtype=np.float32)
    for cid in range(8):
        b, qb = divmod(cid, 4)
        feat[b][:, qb * NQ : (qb + 1) * NQ] = results[cid]["o"]
    return feat.reshape(bz, c, h, w)
